# revision 16
# baseline (speedup 1.0000x reference)
"""Distributed Trainium2 Bass kernel for GQA attention (nn_Attention_27814208209106).

Sharding: 8 cores = 2 batches x 4 KV-head groups.
  Phase 1: x^T via bf16 DMA-transpose (DRAM bounce), per-core q/k/v
           projections (7 q-heads + 1 kv head) + RoPE.
  Phase 2: causal attention in 512-wide T-blocks (k-stationary orientation,
           exp on ScalarE, denominators via ones-matmul), AllGather of each
           block's qkv^T (bf16) within the 4-core batch group overlapped
           with the next block's compute; wo prefetched during attention.
  Phase 3: o-proj per T-block over this core's 896-column output slice.
Host assembles out[b, :, 896*j:896*(j+1)] from core (b, j).

All matmuls in bf16 with f32 PSUM accumulation.
"""

import math
import numpy as np

import concourse.bass as bass
import concourse.mybir as mybir
import concourse.tile as tile
from concourse import bacc
from concourse.bass_utils import run_bass_kernel_spmd

P = 128
FB = 512  # psum free-dim block (f32 psum bank limit)
THETA = 1000000.0

F32 = mybir.dt.float32
BF16 = mybir.dt.bfloat16


class Cfg:
    def __init__(self, T=1024, EMB=3584, NH=28, KVH=4, HD=128):
        self.T, self.EMB, self.NH, self.KVH, self.HD = T, EMB, NH, KVH, HD
        self.GQ = NH // KVH          # q heads per kv head (7)
        self.HG = self.GQ * HD       # per-core q width (896)
        self.NHD = NH * HD           # full qkv width (3584)
        self.EO = EMB // 4           # o-proj output slice per core (896)
        self.KT = EMB // P           # contraction tiles (28)
        self.TT = T // P             # token tiles (8)
        self.NB = (T + FB - 1) // FB  # 512-blocks of T
        self.scale = HD ** -0.5


def _t_blocks(cfg):
    """[(t0, w)] 512-aligned blocks covering [0, T)."""
    return [(b * FB, min(cfg.T, (b + 1) * FB) - b * FB) for b in range(cfg.NB)]


AB = 256  # attention / AllGather chunk width


def _a_chunks(cfg):
    """[(t0, w)] AB-aligned chunks covering [0, T)."""
    n = (cfg.T + AB - 1) // AB
    return [(c * AB, min(cfg.T, (c + 1) * AB) - c * AB) for c in range(n)]


def build_kernel(cfg: Cfg):
    nc = bacc.Bacc(
        "TRN2",
        target_bir_lowering=False,
        debug=False,
        enable_asserts=False,
        num_devices=8,
    )

    xb = nc.dram_tensor("xb", [cfg.EMB, cfg.T], BF16, kind="ExternalInput").ap()
    wq_s = nc.dram_tensor("wq_s", [cfg.GQ * cfg.EMB, cfg.HD], BF16, kind="ExternalInput").ap()
    wk_s = nc.dram_tensor("wk_s", [cfg.EMB, cfg.HD], BF16, kind="ExternalInput").ap()
    wv_s = nc.dram_tensor("wv_s", [cfg.EMB, cfg.HD], BF16, kind="ExternalInput").ap()
    wo_s = nc.dram_tensor("wo_s", [cfg.NHD, cfg.EO], BF16, kind="ExternalInput").ap()
    cosT = nc.dram_tensor("cosT", [cfg.HD // 2, cfg.T], F32, kind="ExternalInput").ap()
    sinT = nc.dram_tensor("sinT", [cfg.HD // 2, cfg.T], F32, kind="ExternalInput").ap()
    o_s = nc.dram_tensor("o_s", [cfg.T, cfg.EO], F32, kind="ExternalOutput").ap()

    with tile.TileContext(nc) as tc:
        _body(tc, cfg, xb, wq_s, wk_s, wv_s, wo_s, cosT, sinT, o_s)

    nc.compile()
    return nc


def _body(tc, cfg, xb, wq_s, wk_s, wv_s, wo_s, cosT, sinT, o_s):
    nc = tc.nc
    H2 = cfg.HD // 2
    tblocks = _t_blocks(cfg)
    KO = 4 * cfg.GQ  # o-proj contraction tiles (28)
    eblocks = [(e * FB, min(cfg.EO, (e + 1) * FB) - e * FB)
               for e in range((cfg.EO + FB - 1) // FB)]

    with (
        tc.tile_pool(name="const", bufs=1) as constp,
        tc.tile_pool(name="qT", bufs=cfg.GQ) as qTp,
        tc.tile_pool(name="kT", bufs=1) as kTp,
        tc.tile_pool(name="vv", bufs=cfg.TT) as vp,
        tc.tile_pool(name="dram", bufs=1, space="DRAM") as dramp,
    ):
        # --- constants ---
        ident = constp.tile([P, P], BF16, name="ident")
        nc.gpsimd.memset(ident, 0.0)
        nc.gpsimd.affine_select(
            out=ident, in_=ident, compare_op=mybir.AluOpType.not_equal,
            fill=1.0, base=0, pattern=[[-1, P]], channel_multiplier=1,
        )
        dmask = constp.tile([P, P], BF16, name="dmask")
        nc.gpsimd.memset(dmask, 1.0)
        nc.gpsimd.affine_select(
            out=dmask, in_=dmask, compare_op=mybir.AluOpType.is_ge,
            fill=0.0, base=0, pattern=[[1, P]], channel_multiplier=-1,
        )
        ones_bf = constp.tile([P, 1], BF16, name="ones_bf")
        nc.vector.memset(ones_bf, 1.0)
        wrm = constp.tile([P, FB], BF16, name="wrm")
        nc.vector.memset(wrm, 0.0)

        qT = [qTp.tile([P, cfg.T], BF16, name=f"qT{h}", tag="qT") for h in range(cfg.GQ)]
        kT = kTp.tile([P, cfg.T], BF16, name="kT")
        vts = [vp.tile([P, cfg.HD], BF16, name=f"v{i}", tag="v") for i in range(cfg.TT)]

        cc_in = [
            dramp.tile([cfg.HG, w], BF16, name=f"cc_in{b}")
            for b, (t0, w) in enumerate(tblocks)
        ]
        cc_out = [
            dramp.tile([4 * cfg.HG, w], BF16, name=f"cc_out{b}")
            for b, (t0, w) in enumerate(tblocks)
        ]

        def attn_head_block(h, tb, pools):
            """Attention for (head h, 512-block tb); writes cc_in[tb] rows of h."""
            plp, psumsp, pmixp, ptp, qkvbp, recp, recbp = pools
            t0b, wb = tblocks[tb]
            si_last = min(cfg.TT - 1, (t0b + wb - 1) // P)
            pts = []
            for si in range(si_last + 1):
                c0 = max(t0b, si * P)
                cw = t0b + wb - c0
                pl = plp.tile([P, FB], F32, name="pl", tag="pl")[:, :cw]
                nc.tensor.matmul(
                    out=pl, lhsT=kT[:, si * P:(si + 1) * P],
                    rhs=qT[h][:, c0:c0 + cw], start=True, stop=True,
                )
                pt = ptp.tile([P, FB], BF16, name="pt", tag="pt")[:, :cw]
                nc.scalar.activation(
                    pt, pl, mybir.ActivationFunctionType.Exp, scale=cfg.scale,
                )
                if si * P >= t0b:
                    nc.gpsimd.tensor_mul(pt[:, 0:P], pt[:, 0:P], dmask)
                pts.append((pt, c0, cw))

            sp = psumsp.tile([1, FB], F32, name="sums", tag="sums")[:, :wb]
            for si, (pt, c0, cw) in enumerate(pts):
                nc.tensor.matmul(
                    out=sp[:, c0 - t0b:c0 - t0b + cw], lhsT=ones_bf, rhs=pt,
                    start=(si == 0), stop=(si == si_last),
                )
            rec = recp.tile([1, FB], F32, name="rec", tag="rec")[:, :wb]
            nc.vector.reciprocal(out=rec, in_=sp)
            recb = recbp.tile([P, FB], F32, name="recb", tag="recb")[:, :wb]
            nc.gpsimd.partition_broadcast(recb, rec)

            pav = pmixp.tile([P, FB], F32, name="pav", tag="pmix")[:, :wb]
            for si, (pt, c0, cw) in enumerate(pts):
                nc.tensor.matmul(
                    out=pav[:, c0 - t0b:c0 - t0b + cw], lhsT=vts[si], rhs=pt,
                    start=(si == 0), stop=(si == si_last),
                )
            qkvb = qkvbp.tile([P, FB], BF16, name="qkvb", tag="qkvb")[:, :wb]
            nc.vector.tensor_mul(qkvb, pav, recb)
            nc.sync.dma_start(cc_in[tb][h * P:(h + 1) * P, :], qkvb)

        def ag(tb):
            nc.gpsimd.collective_compute(
                "AllGather", mybir.AluOpType.bypass,
                replica_groups=[[0, 1, 2, 3], [4, 5, 6, 7]],
                ins=[cc_in[tb].opt()], outs=[cc_out[tb].opt()],
            )

        # ====== L1: projections + first-half attention, then AG0 ======
        with (
            tc.tile_pool(name="rope_cs", bufs=1) as csp,
            tc.tile_pool(name="rtmp", bufs=4) as rtp,
            tc.tile_pool(name="xT", bufs=cfg.KT) as xTp,
            tc.tile_pool(name="wqt", bufs=2) as wqtp,
        ):
            cos_sb = csp.tile([H2, cfg.T], F32, name="cos_sb")
            sin_sb = csp.tile([H2, cfg.T], F32, name="sin_sb")
            nc.sync.dma_start(cos_sb, cosT)
            nc.sync.dma_start(sin_sb, sinT)

            def rope_drain(psum, dst, t0, w):
                c = cos_sb[:, t0:t0 + w]
                s = sin_sb[:, t0:t0 + w]
                p1 = psum[0:H2, :]
                p2 = psum[H2:P, :]
                t1 = rtp.tile([H2, FB], F32, name="t1", tag="rt1")[:, :w]
                t2 = rtp.tile([H2, FB], F32, name="t2", tag="rt2")[:, :w]
                nc.vector.tensor_mul(t1, p1, c)
                nc.vector.tensor_mul(t2, p2, s)
                nc.vector.tensor_sub(dst[0:H2, t0:t0 + w], t1, t2)
                nc.vector.tensor_mul(t1, p2, c)
                nc.vector.tensor_mul(t2, p1, s)
                nc.vector.tensor_add(dst[H2:P, t0:t0 + w], t1, t2)

            # ---- L2a: k/v projections ----
            with (
                tc.tile_pool(name="pwarm", bufs=1, space="PSUM") as pwarmp,
                tc.tile_pool(name="pkv", bufs=2, space="PSUM") as pkvp,
                tc.tile_pool(name="pv", bufs=2, space="PSUM") as pvp,
            ):
                psw = pwarmp.tile([P, FB], F32, name="psw")
                for _ in range(20):
                    nc.tensor.matmul(out=psw, lhsT=ident, rhs=wrm,
                                     start=True, stop=True)

                xTt = [xTp.tile([P, cfg.T], BF16, name=f"xT{k}", tag="xT")
                       for k in range(cfg.KT)]
                wkh, wvh = [], []
                for ke in range(cfg.KT):
                    whk = constp.tile([P, cfg.HD], BF16, name=f"wkh{ke}", tag="wkvh", bufs=2 * cfg.KT)
                    nc.sync.dma_start(whk, wk_s[ke * P:(ke + 1) * P, :])
                    wkh.append(whk)
                    whv = constp.tile([P, cfg.HD], BF16, name=f"wvh{ke}", tag="wkvh", bufs=2 * cfg.KT)
                    nc.sync.dma_start(whv, wv_s[ke * P:(ke + 1) * P, :])
                    wvh.append(whv)
                    nc.sync.dma_start(xTt[ke], xb[ke * P:(ke + 1) * P, :])

                psk = [pkvp.tile([P, FB], F32, name=f"psk{i}", tag="pkv")[:, :w]
                       for i, (t0, w) in enumerate(tblocks)]
                for ke in range(cfg.KT):
                    for i, (t0, w) in enumerate(tblocks):
                        nc.tensor.matmul(
                            out=psk[i], lhsT=wkh[ke], rhs=xTt[ke][:, t0:t0 + w],
                            start=(ke == 0), stop=(ke == cfg.KT - 1),
                        )
                for i, (t0, w) in enumerate(tblocks):
                    rope_drain(psk[i], kT, t0, w)

                for ti in range(cfg.TT):
                    ps = pvp.tile([P, cfg.HD], F32, name="psv", tag="pv")
                    for ke in range(cfg.KT):
                        nc.tensor.matmul(
                            out=ps, lhsT=xTt[ke][:, ti * P:(ti + 1) * P],
                            rhs=wvh[ke],
                            start=(ke == 0), stop=(ke == cfg.KT - 1),
                        )
                    nc.any.tensor_copy(vts[ti], ps)

            # ---- L2b: per-head q-proj + first-half attention ----
            with (
                tc.tile_pool(name="pproj", bufs=3, space="PSUM") as pprojp,
                tc.tile_pool(name="pl", bufs=2, space="PSUM") as plp,
                tc.tile_pool(name="psums", bufs=1, space="PSUM") as psumsp,
                tc.tile_pool(name="pmix", bufs=2, space="PSUM") as pmixp,
                tc.tile_pool(name="pt", bufs=10, space="SBUF") as ptp,
                tc.tile_pool(name="qkvb", bufs=4) as qkvbp,
                tc.tile_pool(name="rec", bufs=4) as recp,
                tc.tile_pool(name="recb", bufs=2) as recbp,
            ):
                poolsA = (plp, psumsp, pmixp, ptp, qkvbp, recp, recbp)
                for h in range(cfg.GQ):
                    wqt = wqtp.tile([P, cfg.KT, cfg.HD], BF16, name="wqt", tag="wqt")
                    nc.sync.dma_start(
                        wqt,
                        wq_s[h * cfg.EMB:(h + 1) * cfg.EMB, :].rearrange(
                            "(ko p) c -> p ko c", p=P),
                    )
                    pss = [pprojp.tile([P, FB], F32, name=f"psq{i}", tag="pproj")[:, :w]
                           for i, (t0, w) in enumerate(tblocks)]
                    for ke in range(cfg.KT):
                        for i, (t0, w) in enumerate(tblocks):
                            nc.tensor.matmul(
                                out=pss[i], lhsT=wqt[:, ke, :],
                                rhs=xTt[ke][:, t0:t0 + w],
                                start=(ke == 0), stop=(ke == cfg.KT - 1),
                            )
                    for i, (t0, w) in enumerate(tblocks):
                        rope_drain(pss[i], qT[h], t0, w)
                    attn_head_block(h, 0, poolsA)
                ag(0)

        # ====== L3: second-half attention, AG1, o-proj ======
        with (
            tc.tile_pool(name="pl2", bufs=2, space="PSUM") as plp2,
            tc.tile_pool(name="psums2", bufs=1, space="PSUM") as psumsp2,
            tc.tile_pool(name="pmix2", bufs=2, space="PSUM") as pmixp2,
            tc.tile_pool(name="po", bufs=3, space="PSUM") as pop,
            tc.tile_pool(name="pt2", bufs=12, space="SBUF") as ptp2,
            tc.tile_pool(name="qkvb2", bufs=4) as qkvbp2,
            tc.tile_pool(name="rec2", bufs=4) as recp2,
            tc.tile_pool(name="recb2", bufs=2) as recbp2,
            tc.tile_pool(name="woh", bufs=KO) as wohp,
            tc.tile_pool(name="qkh", bufs=KO + 6) as qkhp,
            tc.tile_pool(name="osb", bufs=2) as osbp,
        ):
            woh = []
            for kt in range(KO):
                wh = wohp.tile([P, cfg.EO], BF16, name=f"woh{kt}", tag="woh")
                nc.sync.dma_start(wh, wo_s[kt * P:(kt + 1) * P, :])
                woh.append(wh)

            poolsB = (plp2, psumsp2, pmixp2, ptp2, qkvbp2, recp2, recbp2)
            for h in range(cfg.GQ):
                if len(tblocks) > 1:
                    attn_head_block(h, 1, poolsB)
            if len(tblocks) > 1:
                ag(1)

            for tb, (t0b, wb) in enumerate(tblocks):
                qkh = []
                for kt in range(KO):
                    q = qkhp.tile([P, FB], BF16, name=f"qkh{kt}_{tb}", tag="qkh")[:, :wb]
                    nc.sync.dma_start(q, cc_out[tb][kt * P:(kt + 1) * P, :])
                    qkh.append(q)
                for ti in range(wb // P):
                    osb = osbp.tile([P, cfg.EO], F32, name="osb", tag="osb")
                    pos = [
                        pop.tile([P, FB], F32, name=f"po{eb}", tag="po")[:, :ew]
                        for eb, (e0, ew) in enumerate(eblocks)
                    ]
                    for kt in range(KO):
                        for eb, (e0, ew) in enumerate(eblocks):
                            nc.tensor.matmul(
                                out=pos[eb],
                                lhsT=qkh[kt][:, ti * P:(ti + 1) * P],
                                rhs=woh[kt][:, e0:e0 + ew],
                                start=(kt == 0), stop=(kt == KO - 1),
                            )
                    for eb, (e0, ew) in enumerate(eblocks):
                        nc.any.tensor_copy(osb[:, e0:e0 + ew], pos[eb])
                    nc.sync.dma_start(o_s[t0b + ti * P:t0b + (ti + 1) * P, :], osb)


# ======================= host side =======================

_NC_CACHE = {}


def _get_nc(cfg_key=None):
    if cfg_key not in _NC_CACHE:
        _NC_CACHE[cfg_key] = build_kernel(Cfg())
    return _NC_CACHE[cfg_key]


def _rope_tables(segment_ids, cur_ind, T, HD):
    valid = (np.asarray(segment_ids) != 0)
    pos = np.cumsum(valid, axis=-1) - 1 + int(cur_ind)  # [B, T]
    frac = 2.0 * np.arange(HD // 2, dtype=np.float64) / HD
    timescale = THETA ** frac
    ang = pos[..., None].astype(np.float64) / timescale  # [B, T, HD/2]
    cosT = np.transpose(np.cos(ang), (0, 2, 1)).astype(np.float32)  # [B, HD/2, T]
    sinT = np.transpose(np.sin(ang), (0, 2, 1)).astype(np.float32)
    return cosT, sinT


def prepare_in_maps(inputs, cfg=None):
    import ml_dtypes
    bf16 = ml_dtypes.bfloat16
    cfg = cfg or Cfg()
    x = np.asarray(inputs["x"], dtype=np.float32)
    wq = np.asarray(inputs["wq"], dtype=np.float32).astype(bf16)
    wk = np.asarray(inputs["wk"], dtype=np.float32).astype(bf16)
    wv = np.asarray(inputs["wv"], dtype=np.float32).astype(bf16)
    wo = np.asarray(inputs["wo"], dtype=np.float32).astype(bf16)
    seg = np.asarray(inputs["segment_ids"])
    cur = int(np.asarray(inputs["cur_ind"]))

    B, T, EMB = x.shape
    assert (B, T, EMB) == (2, cfg.T, cfg.EMB)
    HG = cfg.HG
    cosT, sinT = _rope_tables(seg, cur, T, cfg.HD)
    xT = np.ascontiguousarray(np.transpose(x, (0, 2, 1))).astype(bf16)  # [B, EMB, T]

    in_maps = []
    for c in range(8):
        b, j = c // 4, c % 4
        wq_j = wq[:, j * HG:(j + 1) * HG].reshape(cfg.EMB, cfg.GQ, cfg.HD)
        wq_j = np.ascontiguousarray(np.transpose(wq_j, (1, 0, 2))).reshape(
            cfg.GQ * cfg.EMB, cfg.HD)
        in_maps.append({
            "xb": xT[b],
            "wq_s": wq_j,
            "wk_s": np.ascontiguousarray(wk[:, j * cfg.HD:(j + 1) * cfg.HD]),
            "wv_s": np.ascontiguousarray(wv[:, j * cfg.HD:(j + 1) * cfg.HD]),
            "wo_s": np.ascontiguousarray(wo[:, j * cfg.EO:(j + 1) * cfg.EO]),
            "cosT": np.ascontiguousarray(cosT[b]),
            "sinT": np.ascontiguousarray(sinT[b]),
        })
    return in_maps


def assemble_out(results, cfg=None):
    cfg = cfg or Cfg()
    out = np.empty((2, cfg.T, cfg.EMB), np.float32)
    for c in range(8):
        b, j = c // 4, c % 4
        out[b, :, j * cfg.EO:(j + 1) * cfg.EO] = results[c]["o_s"]
    return out


def kernel(**inputs):
    cfg = Cfg()
    in_maps = prepare_in_maps(inputs, cfg)
    nc = _get_nc()
    res = run_bass_kernel_spmd(nc, in_maps, core_ids=list(range(8)))
    return assemble_out(res.results, cfg)


# revision 17
# speedup vs baseline: 1.1244x; 1.1244x over previous
"""Distributed Trainium2 Bass kernel for GQA attention (nn_Attention_27814208209106).

Sharding: 8 cores = 2 batches x 4 KV-head groups.
  Phase 1: x^T via bf16 DMA-transpose (DRAM bounce), per-core q/k/v
           projections (7 q-heads + 1 kv head) + RoPE.
  Phase 2: causal attention in 512-wide T-blocks (k-stationary orientation,
           exp on ScalarE, denominators via ones-matmul), AllGather of each
           block's qkv^T (bf16) within the 4-core batch group overlapped
           with the next block's compute; wo prefetched during attention.
  Phase 3: o-proj per T-block over this core's 896-column output slice.
Host assembles out[b, :, 896*j:896*(j+1)] from core (b, j).

All matmuls in bf16 with f32 PSUM accumulation.
"""

import math
import numpy as np

import concourse.bass as bass
import concourse.mybir as mybir
import concourse.tile as tile
from concourse import bacc
from concourse.bass_utils import run_bass_kernel_spmd

P = 128
FB = 512  # psum free-dim block (f32 psum bank limit)
THETA = 1000000.0

F32 = mybir.dt.float32
BF16 = mybir.dt.bfloat16


class Cfg:
    def __init__(self, T=1024, EMB=3584, NH=28, KVH=4, HD=128):
        self.T, self.EMB, self.NH, self.KVH, self.HD = T, EMB, NH, KVH, HD
        self.GQ = NH // KVH          # q heads per kv head (7)
        self.HG = self.GQ * HD       # per-core q width (896)
        self.NHD = NH * HD           # full qkv width (3584)
        self.EO = EMB // 4           # o-proj output slice per core (896)
        self.KT = EMB // P           # contraction tiles (28)
        self.TT = T // P             # token tiles (8)
        self.NB = (T + FB - 1) // FB  # 512-blocks of T
        self.scale = HD ** -0.5


def _t_blocks(cfg):
    """[(t0, w)] 512-aligned blocks covering [0, T)."""
    return [(b * FB, min(cfg.T, (b + 1) * FB) - b * FB) for b in range(cfg.NB)]


AB = 256  # attention / AllGather chunk width


def _a_chunks(cfg):
    """[(t0, w)] AB-aligned chunks covering [0, T)."""
    n = (cfg.T + AB - 1) // AB
    return [(c * AB, min(cfg.T, (c + 1) * AB) - c * AB) for c in range(n)]


def build_kernel(cfg: Cfg):
    nc = bacc.Bacc(
        "TRN2",
        target_bir_lowering=False,
        debug=False,
        enable_asserts=False,
        num_devices=8,
    )

    xb = nc.dram_tensor("xb", [cfg.EMB, cfg.T], BF16, kind="ExternalInput").ap()
    wq_s = nc.dram_tensor("wq_s", [cfg.EMB, cfg.HG], BF16, kind="ExternalInput").ap()
    wk_s = nc.dram_tensor("wk_s", [cfg.EMB, cfg.HD], BF16, kind="ExternalInput").ap()
    wv_s = nc.dram_tensor("wv_s", [cfg.EMB, cfg.HD], BF16, kind="ExternalInput").ap()
    wo_s = nc.dram_tensor("wo_s", [cfg.NHD, cfg.EO], BF16, kind="ExternalInput").ap()
    cosT = nc.dram_tensor("cosT", [cfg.HD // 2, cfg.T], F32, kind="ExternalInput").ap()
    sinT = nc.dram_tensor("sinT", [cfg.HD // 2, cfg.T], F32, kind="ExternalInput").ap()
    o_s = nc.dram_tensor("o_s", [cfg.T, cfg.EO], F32, kind="ExternalOutput").ap()

    with tile.TileContext(nc) as tc:
        _body(tc, cfg, xb, wq_s, wk_s, wv_s, wo_s, cosT, sinT, o_s)

    nc.compile()
    return nc


def _body(tc, cfg, xb, wq_s, wk_s, wv_s, wo_s, cosT, sinT, o_s):
    nc = tc.nc
    H2 = cfg.HD // 2
    tblocks = _t_blocks(cfg)

    with (
        tc.tile_pool(name="const", bufs=1) as constp,
        tc.tile_pool(name="qT", bufs=cfg.GQ) as qTp,
        tc.tile_pool(name="kT", bufs=1) as kTp,
        tc.tile_pool(name="vv", bufs=cfg.TT) as vp,
        tc.tile_pool(name="dram", bufs=1, space="DRAM") as dramp,
    ):
        # --- constants ---
        ident = constp.tile([P, P], BF16, name="ident")
        nc.gpsimd.memset(ident, 0.0)
        nc.gpsimd.affine_select(
            out=ident, in_=ident, compare_op=mybir.AluOpType.not_equal,
            fill=1.0, base=0, pattern=[[-1, P]], channel_multiplier=1,
        )
        # dmask[s, t] = 1 if s <= t else 0  (valid keys in diag tile)
        dmask = constp.tile([P, P], BF16, name="dmask")
        nc.gpsimd.memset(dmask, 1.0)
        nc.gpsimd.affine_select(
            out=dmask, in_=dmask, compare_op=mybir.AluOpType.is_ge,
            fill=0.0, base=0, pattern=[[1, P]], channel_multiplier=-1,
        )
        ones_bf = constp.tile([P, 1], BF16, name="ones_bf")
        nc.vector.memset(ones_bf, 1.0)
        wrm = constp.tile([P, FB], BF16, name="wrm")
        nc.vector.memset(wrm, 0.0)

        qT = [qTp.tile([P, cfg.T], BF16, name=f"qT{h}", tag="qT") for h in range(cfg.GQ)]
        kT = kTp.tile([P, cfg.T], BF16, name="kT")
        vts = [vp.tile([P, cfg.HD], BF16, name=f"v{i}", tag="v") for i in range(cfg.TT)]

        cc_in = [
            dramp.tile([cfg.HG, w], BF16, name=f"cc_in{b}")
            for b, (t0, w) in enumerate(tblocks)
        ]
        cc_out = [
            dramp.tile([4 * cfg.HG, w], BF16, name=f"cc_out{b}")
            for b, (t0, w) in enumerate(tblocks)
        ]

        # ================= Phase 1: x^T + projections =================
        with (
            tc.tile_pool(name="rope_cs", bufs=1) as csp,
            tc.tile_pool(name="xT", bufs=cfg.KT) as xTp,
            tc.tile_pool(name="wqh", bufs=cfg.KT) as wqhp,
            tc.tile_pool(name="wkvh", bufs=2 * cfg.KT) as wkvhp,
            tc.tile_pool(name="pproj", bufs=4, space="PSUM") as pprojp,
            tc.tile_pool(name="pwarm", bufs=1, space="PSUM") as pwarmp,
            tc.tile_pool(name="pv", bufs=2, space="PSUM") as pvp,
            tc.tile_pool(name="rtmp", bufs=4) as rtp,
        ):
            # PE warmup burst (~4us of dense matmuls while DMA streams in)
            psw = pwarmp.tile([P, FB], F32, name="psw")
            for _ in range(20):
                nc.tensor.matmul(out=psw, lhsT=ident, rhs=wrm, start=True, stop=True)

            cos_sb = csp.tile([H2, cfg.T], F32, name="cos_sb")
            sin_sb = csp.tile([H2, cfg.T], F32, name="sin_sb")
            nc.sync.dma_start(cos_sb, cosT)
            nc.sync.dma_start(sin_sb, sinT)

            # x^T / weights arrive pre-transposed + pre-cast (host marshaling);
            # interleave DMAs so the k/v projections can start immediately
            xTt = [xTp.tile([P, cfg.T], BF16, name=f"xT{k}", tag="xT") for k in range(cfg.KT)]
            wkh, wvh, wqh = [], [], []
            for ke in range(cfg.KT):
                whk = wkvhp.tile([P, cfg.HD], BF16, name=f"wkh{ke}", tag="wkvh")
                nc.sync.dma_start(whk, wk_s[ke * P:(ke + 1) * P, :])
                wkh.append(whk)
                whv = wkvhp.tile([P, cfg.HD], BF16, name=f"wvh{ke}", tag="wkvh")
                nc.sync.dma_start(whv, wv_s[ke * P:(ke + 1) * P, :])
                wvh.append(whv)
                nc.sync.dma_start(xTt[ke], xb[ke * P:(ke + 1) * P, :])
            for ke in range(cfg.KT):
                wh = wqhp.tile([P, cfg.HG], BF16, name=f"wqh{ke}", tag="wqh")
                nc.sync.dma_start(wh, wq_s[ke * P:(ke + 1) * P, :])
                wqh.append(wh)

            def rope_drain(psum, dst, t0, w):
                """dst[:, t0:t0+w] = rope(psum) ; psum [128, w] f32."""
                c = cos_sb[:, t0:t0 + w]
                s = sin_sb[:, t0:t0 + w]
                p1 = psum[0:H2, :]
                p2 = psum[H2:P, :]
                t1 = rtp.tile([H2, FB], F32, name="t1", tag="rt1")[:, :w]
                t2 = rtp.tile([H2, FB], F32, name="t2", tag="rt2")[:, :w]
                nc.vector.tensor_mul(t1, p1, c)
                nc.vector.tensor_mul(t2, p2, s)
                nc.vector.tensor_sub(dst[0:H2, t0:t0 + w], t1, t2)
                nc.vector.tensor_mul(t1, p2, c)
                nc.vector.tensor_mul(t2, p1, s)
                nc.vector.tensor_add(dst[H2:P, t0:t0 + w], t1, t2)

            # k projection + rope (first: attention depends on it)
            psk = [pprojp.tile([P, FB], F32, name=f"psk{i}", tag="pproj")[:, :w]
                   for i, (t0, w) in enumerate(tblocks)]
            for ke in range(cfg.KT):
                for i, (t0, w) in enumerate(tblocks):
                    nc.tensor.matmul(
                        out=psk[i], lhsT=wkh[ke], rhs=xTt[ke][:, t0:t0 + w],
                        start=(ke == 0), stop=(ke == cfg.KT - 1),
                    )
            for i, (t0, w) in enumerate(tblocks):
                rope_drain(psk[i], kT, t0, w)

            # v projection: v[ti] = [128 tok, HD] (token-major, no rope)
            for ti in range(cfg.TT):
                ps = pvp.tile([P, cfg.HD], F32, name="psv", tag="pv")
                for ke in range(cfg.KT):
                    nc.tensor.matmul(
                        out=ps, lhsT=xTt[ke][:, ti * P:(ti + 1) * P], rhs=wvh[ke],
                        start=(ke == 0), stop=(ke == cfg.KT - 1),
                    )
                nc.any.tensor_copy(vts[ti], ps)

            # q projection: stationary wq tile reused across all t-blocks
            for h in range(cfg.GQ):
                pss = [pprojp.tile([P, FB], F32, name=f"psq{i}", tag="pproj")[:, :w]
                       for i, (t0, w) in enumerate(tblocks)]
                for ke in range(cfg.KT):
                    for i, (t0, w) in enumerate(tblocks):
                        nc.tensor.matmul(
                            out=pss[i],
                            lhsT=wqh[ke][:, h * P:(h + 1) * P],
                            rhs=xTt[ke][:, t0:t0 + w],
                            start=(ke == 0), stop=(ke == cfg.KT - 1),
                        )
                for i, (t0, w) in enumerate(tblocks):
                    rope_drain(pss[i], qT[h], t0, w)

        # ============ Phase 2+3: attention, AllGather, o-proj ============
        KO = 4 * cfg.GQ  # 28 contraction tiles of the o-proj
        eblocks = [(e * FB, min(cfg.EO, (e + 1) * FB) - e * FB)
                   for e in range((cfg.EO + FB - 1) // FB)]
        with (
            tc.tile_pool(name="pl", bufs=2, space="PSUM") as plp,
            tc.tile_pool(name="psums", bufs=2, space="PSUM") as psumsp,
            tc.tile_pool(name="pmix", bufs=4, space="PSUM") as pmixp,
            tc.tile_pool(name="pt", bufs=2 * cfg.TT, space="SBUF") as ptp,
            tc.tile_pool(name="qkvb", bufs=6) as qkvbp,
            tc.tile_pool(name="rec", bufs=6) as recp,
            tc.tile_pool(name="recb", bufs=4) as recbp,
            tc.tile_pool(name="woh", bufs=KO) as wohp,
            tc.tile_pool(name="qkh", bufs=2 * KO) as qkhp,
            tc.tile_pool(name="osb", bufs=3) as osbp,
        ):
            # prefetch wo during attention (pre-cast bf16)
            woh = []
            for kt in range(KO):
                wh = wohp.tile([P, cfg.EO], BF16, name=f"woh{kt}", tag="woh")
                nc.sync.dma_start(wh, wo_s[kt * P:(kt + 1) * P, :])
                woh.append(wh)

            for tb, (t0b, wb) in enumerate(tblocks):
                si_last = min(cfg.TT - 1, (t0b + wb - 1) // P)
                for h in range(cfg.GQ):
                    pts = []
                    for si in range(si_last + 1):
                        c0 = max(t0b, si * P)
                        cw = t0b + wb - c0
                        pl = plp.tile([P, FB], F32, name="pl", tag="pl")[:, :cw]
                        nc.tensor.matmul(
                            out=pl,
                            lhsT=kT[:, si * P:(si + 1) * P],
                            rhs=qT[h][:, c0:c0 + cw],
                            start=True, stop=True,
                        )
                        pt = ptp.tile([P, FB], BF16, name="pt", tag="pt")[:, :cw]
                        nc.scalar.activation(
                            pt, pl, mybir.ActivationFunctionType.Exp,
                            scale=cfg.scale,
                        )
                        if si * P >= t0b:
                            # diagonal tile: mask invalid (s > t) entries
                            nc.vector.tensor_mul(pt[:, 0:P], pt[:, 0:P], dmask)
                        pts.append((pt, c0, cw))

                    # denominators via ones-matmul over S
                    sp = psumsp.tile([1, FB], F32, name="sums", tag="sums")[:, :wb]
                    for si, (pt, c0, cw) in enumerate(pts):
                        nc.tensor.matmul(
                            out=sp[:, c0 - t0b:c0 - t0b + cw],
                            lhsT=ones_bf, rhs=pt,
                            start=(si == 0), stop=(si == si_last),
                        )
                    rec = recp.tile([1, FB], F32, name="rec", tag="rec")[:, :wb]
                    nc.vector.reciprocal(out=rec, in_=sp)
                    recb = recbp.tile([P, FB], F32, name="recb", tag="recb")[:, :wb]
                    nc.gpsimd.partition_broadcast(recb, rec)

                    # attn @ V (v stationary) + normalize
                    pav = pmixp.tile([P, FB], F32, name="pav", tag="pmix")[:, :wb]
                    for si, (pt, c0, cw) in enumerate(pts):
                        nc.tensor.matmul(
                            out=pav[:, c0 - t0b:c0 - t0b + cw],
                            lhsT=vts[si], rhs=pt,
                            start=(si == 0), stop=(si == si_last),
                        )
                    qkvb = qkvbp.tile([P, FB], BF16, name="qkvb", tag="qkvb")[:, :wb]
                    nc.vector.tensor_mul(qkvb, pav, recb)
                    nc.sync.dma_start(cc_in[tb][h * P:(h + 1) * P, :], qkvb)

                nc.gpsimd.collective_compute(
                    "AllGather",
                    mybir.AluOpType.bypass,
                    replica_groups=[[0, 1, 2, 3], [4, 5, 6, 7]],
                    ins=[cc_in[tb].opt()],
                    outs=[cc_out[tb].opt()],
                )

            # o-proj per block (block 0 overlaps block 1's AllGather)
            for tb, (t0b, wb) in enumerate(tblocks):
                qkh = []
                for kt in range(KO):
                    q = qkhp.tile([P, FB], BF16, name=f"qkh{kt}_{tb}", tag="qkh")[:, :wb]
                    nc.sync.dma_start(q, cc_out[tb][kt * P:(kt + 1) * P, :])
                    qkh.append(q)
                for ti in range(wb // P):
                    osb = osbp.tile([P, cfg.EO], F32, name="osb", tag="osb")
                    pos = [
                        pmixp.tile([P, FB], F32, name=f"po{eb}", tag="pmix")[:, :ew]
                        for eb, (e0, ew) in enumerate(eblocks)
                    ]
                    for kt in range(KO):
                        for eb, (e0, ew) in enumerate(eblocks):
                            nc.tensor.matmul(
                                out=pos[eb],
                                lhsT=qkh[kt][:, ti * P:(ti + 1) * P],
                                rhs=woh[kt][:, e0:e0 + ew],
                                start=(kt == 0), stop=(kt == KO - 1),
                            )
                    for eb, (e0, ew) in enumerate(eblocks):
                        nc.any.tensor_copy(osb[:, e0:e0 + ew], pos[eb])
                    nc.sync.dma_start(o_s[t0b + ti * P:t0b + (ti + 1) * P, :], osb)


# ======================= host side =======================

_NC_CACHE = {}


def _get_nc(cfg_key=None):
    if cfg_key not in _NC_CACHE:
        _NC_CACHE[cfg_key] = build_kernel(Cfg())
    return _NC_CACHE[cfg_key]


def _rope_tables(segment_ids, cur_ind, T, HD):
    valid = (np.asarray(segment_ids) != 0)
    pos = np.cumsum(valid, axis=-1) - 1 + int(cur_ind)  # [B, T]
    frac = 2.0 * np.arange(HD // 2, dtype=np.float64) / HD
    timescale = THETA ** frac
    ang = pos[..., None].astype(np.float64) / timescale  # [B, T, HD/2]
    cosT = np.transpose(np.cos(ang), (0, 2, 1)).astype(np.float32)  # [B, HD/2, T]
    sinT = np.transpose(np.sin(ang), (0, 2, 1)).astype(np.float32)
    return cosT, sinT


def prepare_in_maps(inputs, cfg=None):
    import ml_dtypes
    bf16 = ml_dtypes.bfloat16
    cfg = cfg or Cfg()
    x = np.asarray(inputs["x"], dtype=np.float32)
    wq = np.asarray(inputs["wq"], dtype=np.float32).astype(bf16)
    wk = np.asarray(inputs["wk"], dtype=np.float32).astype(bf16)
    wv = np.asarray(inputs["wv"], dtype=np.float32).astype(bf16)
    wo = np.asarray(inputs["wo"], dtype=np.float32).astype(bf16)
    seg = np.asarray(inputs["segment_ids"])
    cur = int(np.asarray(inputs["cur_ind"]))

    B, T, EMB = x.shape
    assert (B, T, EMB) == (2, cfg.T, cfg.EMB)
    HG = cfg.HG
    cosT, sinT = _rope_tables(seg, cur, T, cfg.HD)
    xT = np.ascontiguousarray(np.transpose(x, (0, 2, 1))).astype(bf16)  # [B, EMB, T]

    in_maps = []
    for c in range(8):
        b, j = c // 4, c % 4
        in_maps.append({
            "xb": xT[b],
            "wq_s": np.ascontiguousarray(wq[:, j * HG:(j + 1) * HG]),
            "wk_s": np.ascontiguousarray(wk[:, j * cfg.HD:(j + 1) * cfg.HD]),
            "wv_s": np.ascontiguousarray(wv[:, j * cfg.HD:(j + 1) * cfg.HD]),
            "wo_s": np.ascontiguousarray(wo[:, j * cfg.EO:(j + 1) * cfg.EO]),
            "cosT": np.ascontiguousarray(cosT[b]),
            "sinT": np.ascontiguousarray(sinT[b]),
        })
    return in_maps


def assemble_out(results, cfg=None):
    cfg = cfg or Cfg()
    out = np.empty((2, cfg.T, cfg.EMB), np.float32)
    for c in range(8):
        b, j = c // 4, c % 4
        out[b, :, j * cfg.EO:(j + 1) * cfg.EO] = results[c]["o_s"]
    return out


def kernel(**inputs):
    cfg = Cfg()
    in_maps = prepare_in_maps(inputs, cfg)
    nc = _get_nc()
    res = run_bass_kernel_spmd(nc, in_maps, core_ids=list(range(8)))
    return assemble_out(res.results, cfg)


# revision 20
# speedup vs baseline: 1.1539x; 1.0262x over previous
"""Distributed Trainium2 Bass kernel for GQA attention (nn_Attention_27814208209106).

Sharding: 8 cores = 2 batches x 4 KV-head groups.
  Phase 1: x^T via bf16 DMA-transpose (DRAM bounce), per-core q/k/v
           projections (7 q-heads + 1 kv head) + RoPE.
  Phase 2: causal attention in 512-wide T-blocks (k-stationary orientation,
           exp on ScalarE, denominators via ones-matmul), AllGather of each
           block's qkv^T (bf16) within the 4-core batch group overlapped
           with the next block's compute; wo prefetched during attention.
  Phase 3: o-proj per T-block over this core's 896-column output slice.
Host assembles out[b, :, 896*j:896*(j+1)] from core (b, j).

All matmuls in bf16 with f32 PSUM accumulation.
"""

import math
import numpy as np

import concourse.bass as bass
import concourse.mybir as mybir
import concourse.tile as tile
from concourse import bacc
from concourse.bass_utils import run_bass_kernel_spmd

P = 128
FB = 512  # psum free-dim block (f32 psum bank limit)
THETA = 1000000.0

F32 = mybir.dt.float32
BF16 = mybir.dt.bfloat16


class Cfg:
    def __init__(self, T=1024, EMB=3584, NH=28, KVH=4, HD=128):
        self.T, self.EMB, self.NH, self.KVH, self.HD = T, EMB, NH, KVH, HD
        self.GQ = NH // KVH          # q heads per kv head (7)
        self.HG = self.GQ * HD       # per-core q width (896)
        self.NHD = NH * HD           # full qkv width (3584)
        self.EO = EMB // 4           # o-proj output slice per core (896)
        self.KT = EMB // P           # contraction tiles (28)
        self.TT = T // P             # token tiles (8)
        self.NB = (T + FB - 1) // FB  # 512-blocks of T
        self.scale = HD ** -0.5


def _t_blocks(cfg):
    """[(t0, w)] 512-aligned blocks covering [0, T)."""
    return [(b * FB, min(cfg.T, (b + 1) * FB) - b * FB) for b in range(cfg.NB)]


AB = 256  # attention / AllGather chunk width


def _a_chunks(cfg):
    """[(t0, w)] AB-aligned chunks covering [0, T)."""
    n = (cfg.T + AB - 1) // AB
    return [(c * AB, min(cfg.T, (c + 1) * AB) - c * AB) for c in range(n)]


def build_kernel(cfg: Cfg):
    nc = bacc.Bacc(
        "TRN2",
        target_bir_lowering=False,
        debug=False,
        enable_asserts=False,
        num_devices=8,
    )

    xb = nc.dram_tensor("xb", [cfg.EMB, cfg.T], BF16, kind="ExternalInput").ap()
    wq_s = nc.dram_tensor("wq_s", [cfg.EMB, cfg.HG], BF16, kind="ExternalInput").ap()
    wk_s = nc.dram_tensor("wk_s", [cfg.EMB, cfg.HD], BF16, kind="ExternalInput").ap()
    wv_s = nc.dram_tensor("wv_s", [cfg.EMB, cfg.HD], BF16, kind="ExternalInput").ap()
    wo_s = nc.dram_tensor("wo_s", [cfg.NHD, cfg.EO], BF16, kind="ExternalInput").ap()
    cosT = nc.dram_tensor("cosT", [cfg.HD // 2, cfg.T], F32, kind="ExternalInput").ap()
    sinT = nc.dram_tensor("sinT", [cfg.HD // 2, cfg.T], F32, kind="ExternalInput").ap()
    o_s = nc.dram_tensor("o_s", [cfg.T, cfg.EO], F32, kind="ExternalOutput").ap()

    with tile.TileContext(nc) as tc:
        _body(tc, cfg, xb, wq_s, wk_s, wv_s, wo_s, cosT, sinT, o_s)

    nc.compile()
    return nc


def _body(tc, cfg, xb, wq_s, wk_s, wv_s, wo_s, cosT, sinT, o_s):
    nc = tc.nc
    H2 = cfg.HD // 2
    tblocks = _t_blocks(cfg)

    with (
        tc.tile_pool(name="const", bufs=1) as constp,
        tc.tile_pool(name="qT", bufs=cfg.GQ) as qTp,
        tc.tile_pool(name="kT", bufs=1) as kTp,
        tc.tile_pool(name="vv", bufs=cfg.TT) as vp,
        tc.tile_pool(name="dram", bufs=1, space="DRAM") as dramp,
    ):
        # --- constants ---
        ident = constp.tile([P, P], BF16, name="ident")
        nc.gpsimd.memset(ident, 0.0)
        nc.gpsimd.affine_select(
            out=ident, in_=ident, compare_op=mybir.AluOpType.not_equal,
            fill=1.0, base=0, pattern=[[-1, P]], channel_multiplier=1,
        )
        # dmask[s, t] = 1 if s <= t else 0  (valid keys in diag tile)
        dmask = constp.tile([P, P], BF16, name="dmask")
        nc.gpsimd.memset(dmask, 1.0)
        nc.gpsimd.affine_select(
            out=dmask, in_=dmask, compare_op=mybir.AluOpType.is_ge,
            fill=0.0, base=0, pattern=[[1, P]], channel_multiplier=-1,
        )
        ones_bf = constp.tile([P, 1], BF16, name="ones_bf")
        nc.vector.memset(ones_bf, 1.0)
        wrm = constp.tile([P, FB], BF16, name="wrm")
        nc.vector.memset(wrm, 0.0)

        qT = [qTp.tile([P, cfg.T], BF16, name=f"qT{h}", tag="qT") for h in range(cfg.GQ)]
        kT = kTp.tile([P, cfg.T], BF16, name="kT")
        vts = [vp.tile([P, cfg.HD], BF16, name=f"v{i}", tag="v") for i in range(cfg.TT)]

        cc_in = [
            dramp.tile([cfg.HG, w], BF16, name=f"cc_in{b}")
            for b, (t0, w) in enumerate(tblocks)
        ]
        cc_out = [
            dramp.tile([4 * cfg.HG, w], BF16, name=f"cc_out{b}")
            for b, (t0, w) in enumerate(tblocks)
        ]

        # ================= Phase 1: x^T + projections =================
        with (
            tc.tile_pool(name="rope_cs", bufs=1) as csp,
            tc.tile_pool(name="xT", bufs=cfg.KT) as xTp,
            tc.tile_pool(name="wqh", bufs=cfg.KT) as wqhp,
            tc.tile_pool(name="wkvh", bufs=2 * cfg.KT) as wkvhp,
            tc.tile_pool(name="pproj", bufs=4, space="PSUM") as pprojp,
            tc.tile_pool(name="pwarm", bufs=1, space="PSUM") as pwarmp,
            tc.tile_pool(name="pv", bufs=2, space="PSUM") as pvp,
            tc.tile_pool(name="rtmp", bufs=4) as rtp,
        ):
            # PE warmup burst (~4us of dense matmuls while DMA streams in)
            psw = pwarmp.tile([P, FB], F32, name="psw")
            for _ in range(20):
                nc.tensor.matmul(out=psw, lhsT=ident, rhs=wrm, start=True, stop=True)

            cos_sb = csp.tile([H2, cfg.T], F32, name="cos_sb")
            sin_sb = csp.tile([H2, cfg.T], F32, name="sin_sb")
            nc.sync.dma_start(cos_sb, cosT)
            nc.sync.dma_start(sin_sb, sinT)

            # x^T / weights arrive pre-transposed + pre-cast (host marshaling);
            # interleave DMAs so the k/v projections can start immediately
            xTt = [xTp.tile([P, cfg.T], BF16, name=f"xT{k}", tag="xT") for k in range(cfg.KT)]
            wkh, wvh, wqh = [], [], []
            for ke in range(cfg.KT):
                whk = wkvhp.tile([P, cfg.HD], BF16, name=f"wkh{ke}", tag="wkvh")
                nc.sync.dma_start(whk, wk_s[ke * P:(ke + 1) * P, :])
                wkh.append(whk)
                whv = wkvhp.tile([P, cfg.HD], BF16, name=f"wvh{ke}", tag="wkvh")
                nc.sync.dma_start(whv, wv_s[ke * P:(ke + 1) * P, :])
                wvh.append(whv)
                nc.sync.dma_start(xTt[ke], xb[ke * P:(ke + 1) * P, :])
            for ke in range(cfg.KT):
                wh = wqhp.tile([P, cfg.HG], BF16, name=f"wqh{ke}", tag="wqh")
                nc.sync.dma_start(wh, wq_s[ke * P:(ke + 1) * P, :])
                wqh.append(wh)

            def rope_drain(psum, dst, t0, w):
                """dst[:, t0:t0+w] = rope(psum) ; psum [128, w] f32."""
                c = cos_sb[:, t0:t0 + w]
                s = sin_sb[:, t0:t0 + w]
                p1 = psum[0:H2, :]
                p2 = psum[H2:P, :]
                t1 = rtp.tile([H2, FB], F32, name="t1", tag="rt1")[:, :w]
                t2 = rtp.tile([H2, FB], F32, name="t2", tag="rt2")[:, :w]
                nc.vector.tensor_mul(t1, p1, c)
                nc.vector.tensor_mul(t2, p2, s)
                nc.vector.tensor_sub(dst[0:H2, t0:t0 + w], t1, t2)
                nc.vector.tensor_mul(t1, p2, c)
                nc.vector.tensor_mul(t2, p1, s)
                nc.vector.tensor_add(dst[H2:P, t0:t0 + w], t1, t2)

            # k projection + rope (first: attention depends on it)
            psk = [pprojp.tile([P, FB], F32, name=f"psk{i}", tag="pproj")[:, :w]
                   for i, (t0, w) in enumerate(tblocks)]
            for ke in range(cfg.KT):
                for i, (t0, w) in enumerate(tblocks):
                    nc.tensor.matmul(
                        out=psk[i], lhsT=wkh[ke], rhs=xTt[ke][:, t0:t0 + w],
                        start=(ke == 0), stop=(ke == cfg.KT - 1),
                    )
            for i, (t0, w) in enumerate(tblocks):
                rope_drain(psk[i], kT, t0, w)

            # v projection: v[ti] = [128 tok, HD] (token-major, no rope)
            for ti in range(cfg.TT):
                ps = pvp.tile([P, cfg.HD], F32, name="psv", tag="pv")
                for ke in range(cfg.KT):
                    nc.tensor.matmul(
                        out=ps, lhsT=xTt[ke][:, ti * P:(ti + 1) * P], rhs=wvh[ke],
                        start=(ke == 0), stop=(ke == cfg.KT - 1),
                    )
                nc.any.tensor_copy(vts[ti], ps)

            # q projection: stationary wq tile reused across all t-blocks
            for h in range(cfg.GQ):
                pss = [pprojp.tile([P, FB], F32, name=f"psq{i}", tag="pproj")[:, :w]
                       for i, (t0, w) in enumerate(tblocks)]
                for ke in range(cfg.KT):
                    for i, (t0, w) in enumerate(tblocks):
                        nc.tensor.matmul(
                            out=pss[i],
                            lhsT=wqh[ke][:, h * P:(h + 1) * P],
                            rhs=xTt[ke][:, t0:t0 + w],
                            start=(ke == 0), stop=(ke == cfg.KT - 1),
                        )
                for i, (t0, w) in enumerate(tblocks):
                    rope_drain(pss[i], qT[h], t0, w)

        # ============ Phase 2+3: attention, AllGather, o-proj ============
        KO = 4 * cfg.GQ  # 28 contraction tiles of the o-proj
        eblocks = [(e * FB, min(cfg.EO, (e + 1) * FB) - e * FB)
                   for e in range((cfg.EO + FB - 1) // FB)]
        with (
            tc.tile_pool(name="pl", bufs=2, space="PSUM") as plp,
            tc.tile_pool(name="psums", bufs=2, space="PSUM") as psumsp,
            tc.tile_pool(name="pmix", bufs=4, space="PSUM") as pmixp,
            tc.tile_pool(name="pt", bufs=18, space="SBUF") as ptp,
            tc.tile_pool(name="qkvb", bufs=8) as qkvbp,
            tc.tile_pool(name="rec", bufs=6) as recp,
            tc.tile_pool(name="recb", bufs=6) as recbp,
            tc.tile_pool(name="woh", bufs=KO) as wohp,
            tc.tile_pool(name="qkh", bufs=2 * KO) as qkhp,
            tc.tile_pool(name="osb", bufs=3) as osbp,
        ):
            # prefetch wo during attention (pre-cast bf16)
            woh = []
            for kt in range(KO):
                wh = wohp.tile([P, cfg.EO], BF16, name=f"woh{kt}", tag="woh")
                nc.sync.dma_start(wh, wo_s[kt * P:(kt + 1) * P, :])
                woh.append(wh)

            for tb, (t0b, wb) in enumerate(tblocks):
                si_last = min(cfg.TT - 1, (t0b + wb - 1) // P)
                for h in range(cfg.GQ):
                    pts = []
                    for si in range(si_last + 1):
                        c0 = max(t0b, si * P)
                        cw = t0b + wb - c0
                        pl = plp.tile([P, FB], F32, name="pl", tag="pl")[:, :cw]
                        nc.tensor.matmul(
                            out=pl,
                            lhsT=kT[:, si * P:(si + 1) * P],
                            rhs=qT[h][:, c0:c0 + cw],
                            start=True, stop=True,
                        )
                        pt = ptp.tile([P, FB], BF16, name="pt", tag="pt")[:, :cw]
                        nc.scalar.activation(
                            pt, pl, mybir.ActivationFunctionType.Exp,
                            scale=cfg.scale,
                        )
                        if si * P >= t0b:
                            # diagonal tile: mask invalid (s > t) entries
                            nc.vector.tensor_mul(pt[:, 0:P], pt[:, 0:P], dmask)
                        pts.append((pt, c0, cw))

                    # denominators via ones-matmul over S
                    sp = psumsp.tile([1, FB], F32, name="sums", tag="sums")[:, :wb]
                    for si, (pt, c0, cw) in enumerate(pts):
                        nc.tensor.matmul(
                            out=sp[:, c0 - t0b:c0 - t0b + cw],
                            lhsT=ones_bf, rhs=pt,
                            start=(si == 0), stop=(si == si_last),
                        )
                    rec = recp.tile([1, FB], F32, name="rec", tag="rec")[:, :wb]
                    nc.vector.reciprocal(out=rec, in_=sp)
                    recb = recbp.tile([P, FB], F32, name="recb", tag="recb")[:, :wb]
                    nc.gpsimd.partition_broadcast(recb, rec)

                    # attn @ V (v stationary) + normalize
                    pav = pmixp.tile([P, FB], F32, name="pav", tag="pmix")[:, :wb]
                    for si, (pt, c0, cw) in enumerate(pts):
                        nc.tensor.matmul(
                            out=pav[:, c0 - t0b:c0 - t0b + cw],
                            lhsT=vts[si], rhs=pt,
                            start=(si == 0), stop=(si == si_last),
                        )
                    qkvb = qkvbp.tile([P, FB], BF16, name="qkvb", tag="qkvb")[:, :wb]
                    nc.vector.tensor_mul(qkvb, pav, recb)
                    nc.sync.dma_start(cc_in[tb][h * P:(h + 1) * P, :], qkvb)

                nc.gpsimd.collective_compute(
                    "AllGather",
                    mybir.AluOpType.bypass,
                    replica_groups=[[0, 1, 2, 3], [4, 5, 6, 7]],
                    ins=[cc_in[tb].opt()],
                    outs=[cc_out[tb].opt()],
                )

            # o-proj per block (block 0 overlaps block 1's AllGather)
            for tb, (t0b, wb) in enumerate(tblocks):
                qkh = []
                for kt in range(KO):
                    q = qkhp.tile([P, FB], BF16, name=f"qkh{kt}_{tb}", tag="qkh")[:, :wb]
                    nc.sync.dma_start(q, cc_out[tb][kt * P:(kt + 1) * P, :])
                    qkh.append(q)
                for ti in range(wb // P):
                    osb = osbp.tile([P, cfg.EO], F32, name="osb", tag="osb")
                    pos = [
                        pmixp.tile([P, FB], F32, name=f"po{eb}", tag="pmix")[:, :ew]
                        for eb, (e0, ew) in enumerate(eblocks)
                    ]
                    for kt in range(KO):
                        for eb, (e0, ew) in enumerate(eblocks):
                            nc.tensor.matmul(
                                out=pos[eb],
                                lhsT=qkh[kt][:, ti * P:(ti + 1) * P],
                                rhs=woh[kt][:, e0:e0 + ew],
                                start=(kt == 0), stop=(kt == KO - 1),
                            )
                    for eb, (e0, ew) in enumerate(eblocks):
                        nc.any.tensor_copy(osb[:, e0:e0 + ew], pos[eb])
                    nc.sync.dma_start(o_s[t0b + ti * P:t0b + (ti + 1) * P, :], osb)


# ======================= host side =======================

_NC_CACHE = {}


def _get_nc(cfg_key=None):
    if cfg_key not in _NC_CACHE:
        _NC_CACHE[cfg_key] = build_kernel(Cfg())
    return _NC_CACHE[cfg_key]


def _rope_tables(segment_ids, cur_ind, T, HD):
    valid = (np.asarray(segment_ids) != 0)
    pos = np.cumsum(valid, axis=-1) - 1 + int(cur_ind)  # [B, T]
    frac = 2.0 * np.arange(HD // 2, dtype=np.float64) / HD
    timescale = THETA ** frac
    ang = pos[..., None].astype(np.float64) / timescale  # [B, T, HD/2]
    cosT = np.transpose(np.cos(ang), (0, 2, 1)).astype(np.float32)  # [B, HD/2, T]
    sinT = np.transpose(np.sin(ang), (0, 2, 1)).astype(np.float32)
    return cosT, sinT


def prepare_in_maps(inputs, cfg=None):
    import ml_dtypes
    bf16 = ml_dtypes.bfloat16
    cfg = cfg or Cfg()
    x = np.asarray(inputs["x"], dtype=np.float32)
    wq = np.asarray(inputs["wq"], dtype=np.float32).astype(bf16)
    wk = np.asarray(inputs["wk"], dtype=np.float32).astype(bf16)
    wv = np.asarray(inputs["wv"], dtype=np.float32).astype(bf16)
    wo = np.asarray(inputs["wo"], dtype=np.float32).astype(bf16)
    seg = np.asarray(inputs["segment_ids"])
    cur = int(np.asarray(inputs["cur_ind"]))

    B, T, EMB = x.shape
    assert (B, T, EMB) == (2, cfg.T, cfg.EMB)
    HG = cfg.HG
    cosT, sinT = _rope_tables(seg, cur, T, cfg.HD)
    xT = np.ascontiguousarray(np.transpose(x, (0, 2, 1))).astype(bf16)  # [B, EMB, T]

    in_maps = []
    for c in range(8):
        b, j = c // 4, c % 4
        in_maps.append({
            "xb": xT[b],
            "wq_s": np.ascontiguousarray(wq[:, j * HG:(j + 1) * HG]),
            "wk_s": np.ascontiguousarray(wk[:, j * cfg.HD:(j + 1) * cfg.HD]),
            "wv_s": np.ascontiguousarray(wv[:, j * cfg.HD:(j + 1) * cfg.HD]),
            "wo_s": np.ascontiguousarray(wo[:, j * cfg.EO:(j + 1) * cfg.EO]),
            "cosT": np.ascontiguousarray(cosT[b]),
            "sinT": np.ascontiguousarray(sinT[b]),
        })
    return in_maps


def assemble_out(results, cfg=None):
    cfg = cfg or Cfg()
    out = np.empty((2, cfg.T, cfg.EMB), np.float32)
    for c in range(8):
        b, j = c // 4, c % 4
        out[b, :, j * cfg.EO:(j + 1) * cfg.EO] = results[c]["o_s"]
    return out


def kernel(**inputs):
    cfg = Cfg()
    in_maps = prepare_in_maps(inputs, cfg)
    nc = _get_nc()
    res = run_bass_kernel_spmd(nc, in_maps, core_ids=list(range(8)))
    return assemble_out(res.results, cfg)


# revision 21
# speedup vs baseline: 1.1636x; 1.0084x over previous
"""Distributed Trainium2 Bass kernel for GQA attention (nn_Attention_27814208209106).

Sharding: 8 cores = 2 batches x 4 KV-head groups.
  Phase 1: x^T via bf16 DMA-transpose (DRAM bounce), per-core q/k/v
           projections (7 q-heads + 1 kv head) + RoPE.
  Phase 2: causal attention in 512-wide T-blocks (k-stationary orientation,
           exp on ScalarE, denominators via ones-matmul), AllGather of each
           block's qkv^T (bf16) within the 4-core batch group overlapped
           with the next block's compute; wo prefetched during attention.
  Phase 3: o-proj per T-block over this core's 896-column output slice.
Host assembles out[b, :, 896*j:896*(j+1)] from core (b, j).

All matmuls in bf16 with f32 PSUM accumulation.
"""

import math
import numpy as np

import concourse.bass as bass
import concourse.mybir as mybir
import concourse.tile as tile
from concourse import bacc
from concourse.bass_utils import run_bass_kernel_spmd

P = 128
FB = 512  # psum free-dim block (f32 psum bank limit)
THETA = 1000000.0

F32 = mybir.dt.float32
BF16 = mybir.dt.bfloat16


class Cfg:
    def __init__(self, T=1024, EMB=3584, NH=28, KVH=4, HD=128):
        self.T, self.EMB, self.NH, self.KVH, self.HD = T, EMB, NH, KVH, HD
        self.GQ = NH // KVH          # q heads per kv head (7)
        self.HG = self.GQ * HD       # per-core q width (896)
        self.NHD = NH * HD           # full qkv width (3584)
        self.EO = EMB // 4           # o-proj output slice per core (896)
        self.KT = EMB // P           # contraction tiles (28)
        self.TT = T // P             # token tiles (8)
        self.NB = (T + FB - 1) // FB  # 512-blocks of T
        self.scale = HD ** -0.5


def _t_blocks(cfg):
    """[(t0, w)] 512-aligned blocks covering [0, T)."""
    return [(b * FB, min(cfg.T, (b + 1) * FB) - b * FB) for b in range(cfg.NB)]


AB = 256  # attention / AllGather chunk width


def _a_chunks(cfg):
    """[(t0, w)] AB-aligned chunks covering [0, T)."""
    n = (cfg.T + AB - 1) // AB
    return [(c * AB, min(cfg.T, (c + 1) * AB) - c * AB) for c in range(n)]


def build_kernel(cfg: Cfg):
    nc = bacc.Bacc(
        "TRN2",
        target_bir_lowering=False,
        debug=False,
        enable_asserts=False,
        num_devices=8,
    )

    xb = nc.dram_tensor("xb", [cfg.EMB, cfg.T], BF16, kind="ExternalInput").ap()
    wq_s = nc.dram_tensor("wq_s", [cfg.EMB, cfg.HG], BF16, kind="ExternalInput").ap()
    wk_s = nc.dram_tensor("wk_s", [cfg.EMB, cfg.HD], BF16, kind="ExternalInput").ap()
    wv_s = nc.dram_tensor("wv_s", [cfg.EMB, cfg.HD], BF16, kind="ExternalInput").ap()
    wo_s = nc.dram_tensor("wo_s", [cfg.NHD, cfg.EO], BF16, kind="ExternalInput").ap()
    cosT = nc.dram_tensor("cosT", [cfg.HD // 2, cfg.T], F32, kind="ExternalInput").ap()
    sinT = nc.dram_tensor("sinT", [cfg.HD // 2, cfg.T], F32, kind="ExternalInput").ap()
    o_s = nc.dram_tensor("o_s", [cfg.T, cfg.EO], F32, kind="ExternalOutput").ap()

    with tile.TileContext(nc) as tc:
        _body(tc, cfg, xb, wq_s, wk_s, wv_s, wo_s, cosT, sinT, o_s)

    nc.compile()
    return nc


def _body(tc, cfg, xb, wq_s, wk_s, wv_s, wo_s, cosT, sinT, o_s):
    nc = tc.nc
    H2 = cfg.HD // 2
    tblocks = _t_blocks(cfg)

    with (
        tc.tile_pool(name="const", bufs=1) as constp,
        tc.tile_pool(name="qT", bufs=cfg.GQ) as qTp,
        tc.tile_pool(name="kT", bufs=1) as kTp,
        tc.tile_pool(name="vv", bufs=cfg.TT) as vp,
        tc.tile_pool(name="dram", bufs=1, space="DRAM") as dramp,
    ):
        # --- constants ---
        ident = constp.tile([P, P], BF16, name="ident")
        nc.gpsimd.memset(ident, 0.0)
        nc.gpsimd.affine_select(
            out=ident, in_=ident, compare_op=mybir.AluOpType.not_equal,
            fill=1.0, base=0, pattern=[[-1, P]], channel_multiplier=1,
        )
        # dmask[s, t] = 1 if s <= t else 0  (valid keys in diag tile)
        dmask = constp.tile([P, P], BF16, name="dmask")
        nc.gpsimd.memset(dmask, 1.0)
        nc.gpsimd.affine_select(
            out=dmask, in_=dmask, compare_op=mybir.AluOpType.is_ge,
            fill=0.0, base=0, pattern=[[1, P]], channel_multiplier=-1,
        )
        ones_bf = constp.tile([P, 1], BF16, name="ones_bf")
        nc.vector.memset(ones_bf, 1.0)
        wrm = constp.tile([P, FB], BF16, name="wrm")
        nc.vector.memset(wrm, 0.0)

        qT = [qTp.tile([P, cfg.T], BF16, name=f"qT{h}", tag="qT") for h in range(cfg.GQ)]
        kT = kTp.tile([P, cfg.T], BF16, name="kT")
        vts = [vp.tile([P, cfg.HD], BF16, name=f"v{i}", tag="v") for i in range(cfg.TT)]

        cc_in = [
            dramp.tile([cfg.HG, w], BF16, name=f"cc_in{b}")
            for b, (t0, w) in enumerate(tblocks)
        ]
        cc_out = [
            dramp.tile([4 * cfg.HG, w], BF16, name=f"cc_out{b}")
            for b, (t0, w) in enumerate(tblocks)
        ]

        # ================= Phase 1: x^T + projections =================
        with (
            tc.tile_pool(name="rope_cs", bufs=1) as csp,
            tc.tile_pool(name="xT", bufs=cfg.KT) as xTp,
            tc.tile_pool(name="wqh", bufs=cfg.KT) as wqhp,
            tc.tile_pool(name="wkvh", bufs=2 * cfg.KT) as wkvhp,
            tc.tile_pool(name="pproj", bufs=4, space="PSUM") as pprojp,
            tc.tile_pool(name="pwarm", bufs=1, space="PSUM") as pwarmp,
            tc.tile_pool(name="pv", bufs=2, space="PSUM") as pvp,
            tc.tile_pool(name="rtmp", bufs=4) as rtp,
        ):
            # PE warmup burst (~4us of dense matmuls while DMA streams in)
            psw = pwarmp.tile([P, FB], F32, name="psw")
            for _ in range(20):
                nc.tensor.matmul(out=psw, lhsT=ident, rhs=wrm, start=True, stop=True)

            cos_sb = csp.tile([H2, cfg.T], F32, name="cos_sb")
            sin_sb = csp.tile([H2, cfg.T], F32, name="sin_sb")
            nc.sync.dma_start(cos_sb, cosT)
            nc.sync.dma_start(sin_sb, sinT)

            # x^T / weights arrive pre-transposed + pre-cast (host marshaling);
            # interleave DMAs so the k/v projections can start immediately
            xTt = [xTp.tile([P, cfg.T], BF16, name=f"xT{k}", tag="xT") for k in range(cfg.KT)]
            wkh, wvh, wqh = [], [], []
            for ke in range(cfg.KT):
                whk = wkvhp.tile([P, cfg.HD], BF16, name=f"wkh{ke}", tag="wkvh")
                nc.sync.dma_start(whk, wk_s[ke * P:(ke + 1) * P, :])
                wkh.append(whk)
                whv = wkvhp.tile([P, cfg.HD], BF16, name=f"wvh{ke}", tag="wkvh")
                nc.sync.dma_start(whv, wv_s[ke * P:(ke + 1) * P, :])
                wvh.append(whv)
                nc.sync.dma_start(xTt[ke], xb[ke * P:(ke + 1) * P, :])
            for ke in range(cfg.KT):
                wh = wqhp.tile([P, cfg.HG], BF16, name=f"wqh{ke}", tag="wqh")
                nc.sync.dma_start(wh, wq_s[ke * P:(ke + 1) * P, :])
                wqh.append(wh)

            def rope_drain(psum, dst, t0, w):
                """dst[:, t0:t0+w] = rope(psum) ; psum [128, w] f32."""
                c = cos_sb[:, t0:t0 + w]
                s = sin_sb[:, t0:t0 + w]
                p1 = psum[0:H2, :]
                p2 = psum[H2:P, :]
                t1 = rtp.tile([H2, FB], F32, name="t1", tag="rt1")[:, :w]
                t2 = rtp.tile([H2, FB], F32, name="t2", tag="rt2")[:, :w]
                nc.vector.tensor_mul(t1, p1, c)
                nc.vector.tensor_mul(t2, p2, s)
                nc.vector.tensor_sub(dst[0:H2, t0:t0 + w], t1, t2)
                nc.vector.tensor_mul(t1, p2, c)
                nc.vector.tensor_mul(t2, p1, s)
                nc.vector.tensor_add(dst[H2:P, t0:t0 + w], t1, t2)

            # k projection + rope (first: attention depends on it)
            psk = [pprojp.tile([P, FB], F32, name=f"psk{i}", tag="pproj")[:, :w]
                   for i, (t0, w) in enumerate(tblocks)]
            for ke in range(cfg.KT):
                for i, (t0, w) in enumerate(tblocks):
                    nc.tensor.matmul(
                        out=psk[i], lhsT=wkh[ke], rhs=xTt[ke][:, t0:t0 + w],
                        start=(ke == 0), stop=(ke == cfg.KT - 1),
                    )
            for i, (t0, w) in enumerate(tblocks):
                rope_drain(psk[i], kT, t0, w)

            # v projection: v[ti] = [128 tok, HD] (token-major, no rope)
            for ti in range(cfg.TT):
                ps = pvp.tile([P, cfg.HD], F32, name="psv", tag="pv")
                for ke in range(cfg.KT):
                    nc.tensor.matmul(
                        out=ps, lhsT=xTt[ke][:, ti * P:(ti + 1) * P], rhs=wvh[ke],
                        start=(ke == 0), stop=(ke == cfg.KT - 1),
                    )
                nc.any.tensor_copy(vts[ti], ps)

            # q projection: stationary wq tile reused across all t-blocks
            for h in range(cfg.GQ):
                pss = [pprojp.tile([P, FB], F32, name=f"psq{i}", tag="pproj")[:, :w]
                       for i, (t0, w) in enumerate(tblocks)]
                for ke in range(cfg.KT):
                    for i, (t0, w) in enumerate(tblocks):
                        nc.tensor.matmul(
                            out=pss[i],
                            lhsT=wqh[ke][:, h * P:(h + 1) * P],
                            rhs=xTt[ke][:, t0:t0 + w],
                            start=(ke == 0), stop=(ke == cfg.KT - 1),
                        )
                for i, (t0, w) in enumerate(tblocks):
                    rope_drain(pss[i], qT[h], t0, w)

        # ============ Phase 2+3: attention, AllGather, o-proj ============
        KO = 4 * cfg.GQ  # 28 contraction tiles of the o-proj
        eblocks = [(e * FB, min(cfg.EO, (e + 1) * FB) - e * FB)
                   for e in range((cfg.EO + FB - 1) // FB)]
        with (
            tc.tile_pool(name="pl", bufs=2, space="PSUM") as plp,
            tc.tile_pool(name="psums", bufs=2, space="PSUM") as psumsp,
            tc.tile_pool(name="pmix", bufs=4, space="PSUM") as pmixp,
            tc.tile_pool(name="pt", bufs=18, space="SBUF") as ptp,
            tc.tile_pool(name="qkvb", bufs=8) as qkvbp,
            tc.tile_pool(name="rec", bufs=6) as recp,
            tc.tile_pool(name="recb", bufs=6) as recbp,
            tc.tile_pool(name="woh", bufs=KO) as wohp,
            tc.tile_pool(name="qkh", bufs=2 * KO) as qkhp,
            tc.tile_pool(name="osb", bufs=3) as osbp,
        ):
            # prefetch wo during attention (pre-cast bf16)
            woh = []
            for kt in range(KO):
                wh = wohp.tile([P, cfg.EO], BF16, name=f"woh{kt}", tag="woh")
                nc.sync.dma_start(wh, wo_s[kt * P:(kt + 1) * P, :])
                woh.append(wh)

            def warm_burst(n):
                pw = pmixp.tile([P, FB], F32, name="pwb", tag="pmix")
                for _ in range(n):
                    nc.tensor.matmul(out=pw, lhsT=ident, rhs=wrm,
                                     start=True, stop=True)

            for tb, (t0b, wb) in enumerate(tblocks):
                si_last = min(cfg.TT - 1, (t0b + wb - 1) // P)
                for h in range(cfg.GQ):
                    pts = []
                    for si in range(si_last + 1):
                        c0 = max(t0b, si * P)
                        cw = t0b + wb - c0
                        pl = plp.tile([P, FB], F32, name="pl", tag="pl")[:, :cw]
                        nc.tensor.matmul(
                            out=pl,
                            lhsT=kT[:, si * P:(si + 1) * P],
                            rhs=qT[h][:, c0:c0 + cw],
                            start=True, stop=True,
                        )
                        pt = ptp.tile([P, FB], BF16, name="pt", tag="pt")[:, :cw]
                        nc.scalar.activation(
                            pt, pl, mybir.ActivationFunctionType.Exp,
                            scale=cfg.scale,
                        )
                        if si * P >= t0b:
                            # diagonal tile: mask invalid (s > t) entries
                            nc.vector.tensor_mul(pt[:, 0:P], pt[:, 0:P], dmask)
                        pts.append((pt, c0, cw))

                    # denominators via ones-matmul over S
                    sp = psumsp.tile([1, FB], F32, name="sums", tag="sums")[:, :wb]
                    for si, (pt, c0, cw) in enumerate(pts):
                        nc.tensor.matmul(
                            out=sp[:, c0 - t0b:c0 - t0b + cw],
                            lhsT=ones_bf, rhs=pt,
                            start=(si == 0), stop=(si == si_last),
                        )
                    rec = recp.tile([1, FB], F32, name="rec", tag="rec")[:, :wb]
                    nc.vector.reciprocal(out=rec, in_=sp)
                    recb = recbp.tile([P, FB], F32, name="recb", tag="recb")[:, :wb]
                    nc.gpsimd.partition_broadcast(recb, rec)

                    # attn @ V (v stationary) + normalize
                    pav = pmixp.tile([P, FB], F32, name="pav", tag="pmix")[:, :wb]
                    for si, (pt, c0, cw) in enumerate(pts):
                        nc.tensor.matmul(
                            out=pav[:, c0 - t0b:c0 - t0b + cw],
                            lhsT=vts[si], rhs=pt,
                            start=(si == 0), stop=(si == si_last),
                        )
                    qkvb = qkvbp.tile([P, FB], BF16, name="qkvb", tag="qkvb")[:, :wb]
                    nc.vector.tensor_mul(qkvb, pav, recb)
                    nc.sync.dma_start(cc_in[tb][h * P:(h + 1) * P, :], qkvb)

                nc.gpsimd.collective_compute(
                    "AllGather",
                    mybir.AluOpType.bypass,
                    replica_groups=[[0, 1, 2, 3], [4, 5, 6, 7]],
                    ins=[cc_in[tb].opt()],
                    outs=[cc_out[tb].opt()],
                )
                warm_burst(12)

            # o-proj per block (block 0 overlaps block 1's AllGather)
            for tb, (t0b, wb) in enumerate(tblocks):
                warm_burst(10)
                qkh = []
                for kt in range(KO):
                    q = qkhp.tile([P, FB], BF16, name=f"qkh{kt}_{tb}", tag="qkh")[:, :wb]
                    nc.sync.dma_start(q, cc_out[tb][kt * P:(kt + 1) * P, :])
                    qkh.append(q)
                for ti in range(wb // P):
                    osb = osbp.tile([P, cfg.EO], F32, name="osb", tag="osb")
                    pos = [
                        pmixp.tile([P, FB], F32, name=f"po{eb}", tag="pmix")[:, :ew]
                        for eb, (e0, ew) in enumerate(eblocks)
                    ]
                    for kt in range(KO):
                        for eb, (e0, ew) in enumerate(eblocks):
                            nc.tensor.matmul(
                                out=pos[eb],
                                lhsT=qkh[kt][:, ti * P:(ti + 1) * P],
                                rhs=woh[kt][:, e0:e0 + ew],
                                start=(kt == 0), stop=(kt == KO - 1),
                            )
                    for eb, (e0, ew) in enumerate(eblocks):
                        nc.any.tensor_copy(osb[:, e0:e0 + ew], pos[eb])
                    nc.sync.dma_start(o_s[t0b + ti * P:t0b + (ti + 1) * P, :], osb)


# ======================= host side =======================

_NC_CACHE = {}


def _get_nc(cfg_key=None):
    if cfg_key not in _NC_CACHE:
        _NC_CACHE[cfg_key] = build_kernel(Cfg())
    return _NC_CACHE[cfg_key]


def _rope_tables(segment_ids, cur_ind, T, HD):
    valid = (np.asarray(segment_ids) != 0)
    pos = np.cumsum(valid, axis=-1) - 1 + int(cur_ind)  # [B, T]
    frac = 2.0 * np.arange(HD // 2, dtype=np.float64) / HD
    timescale = THETA ** frac
    ang = pos[..., None].astype(np.float64) / timescale  # [B, T, HD/2]
    cosT = np.transpose(np.cos(ang), (0, 2, 1)).astype(np.float32)  # [B, HD/2, T]
    sinT = np.transpose(np.sin(ang), (0, 2, 1)).astype(np.float32)
    return cosT, sinT


def prepare_in_maps(inputs, cfg=None):
    import ml_dtypes
    bf16 = ml_dtypes.bfloat16
    cfg = cfg or Cfg()
    x = np.asarray(inputs["x"], dtype=np.float32)
    wq = np.asarray(inputs["wq"], dtype=np.float32).astype(bf16)
    wk = np.asarray(inputs["wk"], dtype=np.float32).astype(bf16)
    wv = np.asarray(inputs["wv"], dtype=np.float32).astype(bf16)
    wo = np.asarray(inputs["wo"], dtype=np.float32).astype(bf16)
    seg = np.asarray(inputs["segment_ids"])
    cur = int(np.asarray(inputs["cur_ind"]))

    B, T, EMB = x.shape
    assert (B, T, EMB) == (2, cfg.T, cfg.EMB)
    HG = cfg.HG
    cosT, sinT = _rope_tables(seg, cur, T, cfg.HD)
    xT = np.ascontiguousarray(np.transpose(x, (0, 2, 1))).astype(bf16)  # [B, EMB, T]

    in_maps = []
    for c in range(8):
        b, j = c // 4, c % 4
        in_maps.append({
            "xb": xT[b],
            "wq_s": np.ascontiguousarray(wq[:, j * HG:(j + 1) * HG]),
            "wk_s": np.ascontiguousarray(wk[:, j * cfg.HD:(j + 1) * cfg.HD]),
            "wv_s": np.ascontiguousarray(wv[:, j * cfg.HD:(j + 1) * cfg.HD]),
            "wo_s": np.ascontiguousarray(wo[:, j * cfg.EO:(j + 1) * cfg.EO]),
            "cosT": np.ascontiguousarray(cosT[b]),
            "sinT": np.ascontiguousarray(sinT[b]),
        })
    return in_maps


def assemble_out(results, cfg=None):
    cfg = cfg or Cfg()
    out = np.empty((2, cfg.T, cfg.EMB), np.float32)
    for c in range(8):
        b, j = c // 4, c % 4
        out[b, :, j * cfg.EO:(j + 1) * cfg.EO] = results[c]["o_s"]
    return out


def kernel(**inputs):
    cfg = Cfg()
    in_maps = prepare_in_maps(inputs, cfg)
    nc = _get_nc()
    res = run_bass_kernel_spmd(nc, in_maps, core_ids=list(range(8)))
    return assemble_out(res.results, cfg)
